# revision 10
# baseline (speedup 1.0000x reference)
"""Trainium2 Bass kernel for 4D valid convolution.

x (2,2,32,32,64,64) f32, weight (4,2,3,3,3,3) f32, bias (4,) f32
-> out (2,4,30,30,62,62) f32  (valid cross-correlation + bias)

Strategy: 8 cores = batch(2) x a-quadrant(4). Each core computes
out[b, :, a_sel, :, :, :] from slab x[b, :, a0:a0+10, :, :, :].

TensorE mapping per core:
  K (contraction, partitions) = (ci=2, a-window=10, b-window=6) = 120
  M (psum partitions)         = (co=4, a_out=8, b_out=4) = 128
  N (streamed free dim)       = contiguous (c,d) output pixels, <=512
The weights for the (a,b) window band are prebuilt on host per (k,l)
tap; the 9 (k,l) taps accumulate in PSUM using (c,d)-shifted views of
the same SBUF x tile. float32r matmuls (1 row/cycle at N>=256).
"""

import sys

if "/opt/trn_rl_repo" not in sys.path:
    sys.path.insert(0, "/opt/trn_rl_repo")

import numpy as np

B, CI, CO = 2, 2, 4
A, B2, C, D = 32, 32, 64, 64
AO, BO, CL, DL = 30, 30, 62, 62
K = 3

# per-core a-slab starts; each core computes 8 output a-rows (q=3 overlaps q=2)
A0 = [0, 8, 16, 22]
SA = 10  # a-window (8 outputs + 2 halo)
SB = 6  # b-window per block (4 outputs + 2 halo)
NBB = 8  # b_out blocks: 7 full (4 wide) + 1 last (2 wide)
NCC = 8  # c chunks: 7 full (8 wide) + 1 last (6 wide)

_CACHE = {}


def _build_weights(weight: np.ndarray, bias: np.ndarray):
    """Banded lhsT matrices per (k,l) tap, plus per-partition bias vectors."""
    w = weight.astype(np.float32)

    def banded(sa, n_ao, sb, n_bo):
        # sel[d, o, t] = 1 if d == o + t
        sa_sel = np.zeros((sa, n_ao, K), np.float32)
        for t in range(K):
            for o in range(n_ao):
                sa_sel[o + t, o, t] = 1.0
        sb_sel = np.zeros((sb, n_bo, K), np.float32)
        for t in range(K):
            for o in range(n_bo):
                sb_sel[o + t, o, t] = 1.0
        # lhsT[t=(k,l)][(ci,da,db), (co,ao,bo)]
        out = np.zeros((9, CI * sa * sb, CO * n_ao * n_bo), np.float32)
        for k in range(K):
            for l in range(K):
                wkl = w[:, :, :, :, k, l]  # (co, ci, i, j)
                m = np.einsum("dai,ebj,ocij->cdeoab", sa_sel, sb_sel, wkl)
                out[k * 3 + l] = m.reshape(CI * sa * sb, CO * n_ao * n_bo)
        return np.ascontiguousarray(out)

    w_main = banded(SA, 8, SB, 4)  # (9, 120, 128)
    w_last = banded(SA, 8, 4, 2)  # (9, 80, 64)
    bias_main = np.repeat(bias.astype(np.float32), 32).reshape(128, 1)
    bias_last = np.repeat(bias.astype(np.float32), 16).reshape(64, 1)
    return w_main, w_last, bias_main, bias_last


def _build_program():
    import concourse.bass as bass  # noqa: F401
    import concourse.mybir as mybir
    import concourse.tile as tile
    from concourse import bacc

    f32 = mybir.dt.float32
    f32r = mybir.dt.float32r

    nc = bacc.Bacc("TRN2", target_bir_lowering=False, debug=False, num_devices=8)
    xs = nc.dram_tensor("x_slab", [CI, SA, B2, C, D], f32r, kind="ExternalInput")
    wm = nc.dram_tensor("w_main", [9, 120, 128], f32r, kind="ExternalInput")
    wl = nc.dram_tensor("w_last", [9, 80, 64], f32r, kind="ExternalInput")
    bm = nc.dram_tensor("bias_main", [128, 1], f32, kind="ExternalInput")
    bl = nc.dram_tensor("bias_last", [64, 1], f32, kind="ExternalInput")
    # partition-major blocks: [bb, cc, m, n]; host unscrambles (cheap numpy)
    out = nc.dram_tensor(
        "out_blocks", [NBB, NCC, 128, 8 * DL], f32, kind="ExternalOutput"
    )

    with tile.TileContext(nc) as tc:
        with (
            tc.tile_pool(name="w", bufs=1) as wpool,
            tc.tile_pool(name="rhs", bufs=2) as rpool,
            tc.tile_pool(name="psum", bufs=8, space="PSUM") as ppool,
            tc.tile_pool(name="ot", bufs=4) as opool,
        ):
            w_main_t = wpool.tile([120, 9 * 128], f32r)
            for t in range(9):
                nc.sync.dma_start(w_main_t[:, t * 128 : (t + 1) * 128], wm[t])
            w_last_t = wpool.tile([80, 9 * 64], f32r)
            for t in range(9):
                nc.sync.dma_start(w_last_t[:, t * 64 : (t + 1) * 64], wl[t])
            bias_main_t = wpool.tile([128, 1], f32)
            nc.sync.dma_start(bias_main_t[:], bm[:])
            bias_last_t = wpool.tile([64, 1], f32)
            nc.sync.dma_start(bias_last_t[:], bl[:])

            for bb in range(NBB):
                b0 = bb * 4
                wb = SB if bb < NBB - 1 else 4  # b-window width
                wbo = 4 if bb < NBB - 1 else 2  # b_out width
                P = CI * SA * wb  # 120 or 80
                M = CO * 8 * wbo  # 128 or 64

                rhs_t = rpool.tile([P, C * D], f32r, tag="rhs")
                for ci in range(CI):
                    for a in range(SA):
                        p0 = (ci * SA + a) * wb
                        nc.sync.dma_start(
                            rhs_t[p0 : p0 + wb, :],
                            xs[ci, a, b0 : b0 + wb].rearrange("db c d -> db (c d)"),
                        )
                rhs3 = rhs_t.rearrange("p (c d) -> p c d", c=C)
                w_t = w_main_t if bb < NBB - 1 else w_last_t
                bias_t = bias_main_t if bb < NBB - 1 else bias_last_t

                for cc in range(NCC):
                    c0 = cc * 8
                    wc = 8 if cc < NCC - 1 else 6
                    N = wc * DL
                    ps = ppool.tile([M, N], f32, tag="ps")
                    for t in range(9):
                        k, l = divmod(t, 3)
                        rv = rhs3[:, c0 + k : c0 + k + wc, l : l + DL]
                        nc.tensor.matmul(
                            ps.rearrange("m (c d) -> m c d", c=wc),
                            w_t[:, t * M : (t + 1) * M],
                            rv,
                            start=(t == 0),
                            stop=(t == 8),
                        )
                    ot = opool.tile([M, N], f32, tag="ot")
                    nc.vector.tensor_scalar_add(ot[:], ps[:], bias_t[:])
                    nc.sync.dma_start(out[bb, cc, :M, :N], ot[:])
    nc.compile()
    return nc


def kernel(x: np.ndarray, weight: np.ndarray, bias: np.ndarray) -> np.ndarray:
    from concourse.bass_utils import run_bass_kernel_spmd

    if "nc" not in _CACHE:
        _CACHE["nc"] = _build_program()
    nc = _CACHE["nc"]

    w_main, w_last, bias_main, bias_last = _build_weights(weight, bias)
    x = np.ascontiguousarray(x.astype(np.float32, copy=False))

    in_maps = []
    for core in range(8):
        b, q = divmod(core, 4)
        a0 = A0[q]
        in_maps.append(
            {
                "x_slab": np.ascontiguousarray(x[b, :, a0 : a0 + SA]),
                "w_main": w_main,
                "w_last": w_last,
                "bias_main": bias_main,
                "bias_last": bias_last,
            }
        )

    res = run_bass_kernel_spmd(nc, in_maps, core_ids=list(range(8)))
    _CACHE["last_result"] = res

    out = np.empty((B, CO, AO, BO, CL, DL), np.float32)
    for core in range(8):
        b, q = divmod(core, 4)
        slab = _unscramble(res.results[core]["out_blocks"])  # (4, 8, 30, 62, 62)
        if q < 3:
            out[b, :, 8 * q : 8 * q + 8] = slab
        else:
            out[b, :, 24:30] = slab[:, 2:8]
    return out


def _unscramble(blocks: np.ndarray) -> np.ndarray:
    """[NBB, NCC, 128, 8*62] partition-major blocks -> (4, 8, 30, 62, 62) slab."""
    slab = np.empty((CO, 8, BO, CL, DL), np.float32)
    for bb in range(NBB):
        wbo = 4 if bb < NBB - 1 else 2
        m = CO * 8 * wbo
        for cc in range(NCC):
            wc = 8 if cc < NCC - 1 else 6
            n = wc * DL
            blk = blocks[bb, cc, :m, :n].reshape(CO, 8, wbo, wc, DL)
            slab[:, :, bb * 4 : bb * 4 + wbo, cc * 8 : cc * 8 + wc, :] = blk
    return slab


# revision 17
# speedup vs baseline: 1.2392x; 1.2392x over previous
"""Trainium2 Bass kernel for 4D valid convolution.

x (2,2,32,32,64,64) f32, weight (4,2,3,3,3,3) f32, bias (4,) f32
-> out (2,4,30,30,62,62) f32  (valid cross-correlation + bias)

Strategy: 8 cores = batch(2) x a-quadrant(4). Each core computes
out[b, :, a_sel, :, :, :] from slab x[b, :, a0:a0+10, :, :, :].

TensorE mapping per core:
  K (contraction, partitions) = (ci=2, a-window=10, b-window=6) = 120
  M (psum partitions)         = (co=4, a_out=8, b_out=4) = 128
  N (streamed free dim)       = contiguous (c,d) output pixels, <=512
The weights for the (a,b) window band are prebuilt on host per (k,l)
tap; the 9 (k,l) taps accumulate in PSUM using (c,d)-shifted views of
the same SBUF x tile. float32r matmuls (1 row/cycle at N>=256).
"""

import sys

if "/opt/trn_rl_repo" not in sys.path:
    sys.path.insert(0, "/opt/trn_rl_repo")

import ml_dtypes
import numpy as np

BF16 = ml_dtypes.bfloat16

B, CI, CO = 2, 2, 4
A, B2, C, D = 32, 32, 64, 64
AO, BO, CL, DL = 30, 30, 62, 62
K = 3

# per-core a-slab starts; each core computes 8 output a-rows (q=3 overlaps q=2)
A0 = [0, 8, 16, 22]
SA = 10  # a-window (8 outputs + 2 halo)
SB = 6  # b-window per block (4 outputs + 2 halo)
NBB = 8  # b_out blocks: 7 full (4 wide) + 1 last (2 wide)
NCC = 8  # c chunks: 7 full (8 wide) + 1 last (6 wide)

_CACHE = {}


def _build_weights(weight: np.ndarray, bias: np.ndarray):
    """Banded lhsT matrices per (k,l) tap, plus per-partition bias vectors."""
    w = weight.astype(np.float32)

    def banded(sa, n_ao, sb, n_bo):
        # sel[d, o, t] = 1 if d == o + t
        sa_sel = np.zeros((sa, n_ao, K), np.float32)
        for t in range(K):
            for o in range(n_ao):
                sa_sel[o + t, o, t] = 1.0
        sb_sel = np.zeros((sb, n_bo, K), np.float32)
        for t in range(K):
            for o in range(n_bo):
                sb_sel[o + t, o, t] = 1.0
        # lhsT[t=(k,l)][(ci,da,db), (co,ao,bo)]
        out = np.zeros((9, CI * sa * sb, CO * n_ao * n_bo), np.float32)
        for k in range(K):
            for l in range(K):
                wkl = w[:, :, :, :, k, l]  # (co, ci, i, j)
                m = np.einsum("dai,ebj,ocij->cdeoab", sa_sel, sb_sel, wkl)
                out[k * 3 + l] = m.reshape(CI * sa * sb, CO * n_ao * n_bo)
        return np.ascontiguousarray(out)

    w_main = banded(SA, 8, SB, 4)  # (9, 120, 128)
    w_last = banded(SA, 8, 4, 2)  # (9, 80, 64)
    bias_main = np.repeat(bias.astype(np.float32), 32).reshape(128, 1)
    bias_last = np.repeat(bias.astype(np.float32), 16).reshape(64, 1)
    return w_main, w_last, bias_main, bias_last


def _build_program():
    import concourse.bass as bass  # noqa: F401
    import concourse.mybir as mybir
    import concourse.tile as tile
    from concourse import bacc

    f32 = mybir.dt.float32
    bf16 = mybir.dt.bfloat16

    nc = bacc.Bacc("TRN2", target_bir_lowering=False, debug=False, num_devices=8)
    xs = nc.dram_tensor("x_slab", [CI, SA, B2, C, D], bf16, kind="ExternalInput")
    wm = nc.dram_tensor("w_main", [9, 120, 128], bf16, kind="ExternalInput")
    wl = nc.dram_tensor("w_last", [9, 80, 64], bf16, kind="ExternalInput")
    bm = nc.dram_tensor("bias_main", [128, 1], f32, kind="ExternalInput")
    bl = nc.dram_tensor("bias_last", [64, 1], f32, kind="ExternalInput")
    # partition-major blocks: [bb, cc, m, n]; host unscrambles (cheap numpy)
    out = nc.dram_tensor(
        "out_blocks", [NBB, NCC, 128, 8 * DL], f32, kind="ExternalOutput"
    )

    with tile.TileContext(nc) as tc:
        with (
            tc.tile_pool(name="w", bufs=1) as wpool,
            tc.tile_pool(name="rhs", bufs=2) as rpool,
            tc.tile_pool(name="psum", bufs=8, space="PSUM") as ppool,
            tc.tile_pool(name="ot", bufs=4) as opool,
        ):
            w_main_t = wpool.tile([120, 9 * 128], bf16)
            for t in range(9):
                nc.sync.dma_start(w_main_t[:, t * 128 : (t + 1) * 128], wm[t])
            w_last_t = wpool.tile([80, 9 * 64], bf16)
            for t in range(9):
                nc.sync.dma_start(w_last_t[:, t * 64 : (t + 1) * 64], wl[t])
            bias_main_t = wpool.tile([128, 1], f32)
            nc.sync.dma_start(bias_main_t[:], bm[:])
            bias_last_t = wpool.tile([64, 1], f32)
            nc.sync.dma_start(bias_last_t[:], bl[:])

            for bb in range(NBB):
                b0 = bb * 4
                wb = SB if bb < NBB - 1 else 4  # b-window width
                wbo = 4 if bb < NBB - 1 else 2  # b_out width
                P = CI * SA * wb  # 120 or 80
                M = CO * 8 * wbo  # 128 or 64

                rhs_t = rpool.tile([P, C * D], bf16, tag="rhs")
                for ci in range(CI):
                    for a in range(SA):
                        p0 = (ci * SA + a) * wb
                        nc.sync.dma_start(
                            rhs_t[p0 : p0 + wb, :],
                            xs[ci, a, b0 : b0 + wb].rearrange("db c d -> db (c d)"),
                        )
                rhs3 = rhs_t.rearrange("p (c d) -> p c d", c=C)
                w_t = w_main_t if bb < NBB - 1 else w_last_t
                bias_t = bias_main_t if bb < NBB - 1 else bias_last_t

                for cc in range(NCC):
                    c0 = cc * 8
                    wc = 8 if cc < NCC - 1 else 6
                    N = wc * DL
                    ps = ppool.tile([M, N], f32, tag="ps")
                    for t in range(9):
                        k, l = divmod(t, 3)
                        rv = rhs3[:, c0 + k : c0 + k + wc, l : l + DL]
                        nc.tensor.matmul(
                            ps.rearrange("m (c d) -> m c d", c=wc),
                            w_t[:, t * M : (t + 1) * M],
                            rv,
                            start=(t == 0),
                            stop=(t == 8),
                        )
                    ot = opool.tile([M, N], f32, tag="ot")
                    nc.vector.tensor_scalar_add(ot[:], ps[:], bias_t[:])
                    nc.sync.dma_start(out[bb, cc, :M, :N], ot[:])
    nc.compile()
    return nc


def kernel(x: np.ndarray, weight: np.ndarray, bias: np.ndarray) -> np.ndarray:
    from concourse.bass_utils import run_bass_kernel_spmd

    if "nc" not in _CACHE:
        _CACHE["nc"] = _build_program()
    nc = _CACHE["nc"]

    w_main, w_last, bias_main, bias_last = _build_weights(weight, bias)
    x_bf = x.astype(BF16)
    w_main = w_main.astype(BF16)
    w_last = w_last.astype(BF16)

    in_maps = []
    for core in range(8):
        b, q = divmod(core, 4)
        a0 = A0[q]
        in_maps.append(
            {
                "x_slab": np.ascontiguousarray(x_bf[b, :, a0 : a0 + SA]),
                "w_main": w_main,
                "w_last": w_last,
                "bias_main": bias_main,
                "bias_last": bias_last,
            }
        )

    res = run_bass_kernel_spmd(nc, in_maps, core_ids=list(range(8)))
    _CACHE["last_result"] = res

    out = np.empty((B, CO, AO, BO, CL, DL), np.float32)
    for core in range(8):
        b, q = divmod(core, 4)
        slab = _unscramble(res.results[core]["out_blocks"])  # (4, 8, 30, 62, 62)
        if q < 3:
            out[b, :, 8 * q : 8 * q + 8] = slab
        else:
            out[b, :, 24:30] = slab[:, 2:8]
    return out


def _unscramble(blocks: np.ndarray) -> np.ndarray:
    """[NBB, NCC, 128, 8*62] partition-major blocks -> (4, 8, 30, 62, 62) slab."""
    slab = np.empty((CO, 8, BO, CL, DL), np.float32)
    for bb in range(NBB):
        wbo = 4 if bb < NBB - 1 else 2
        m = CO * 8 * wbo
        for cc in range(NCC):
            wc = 8 if cc < NCC - 1 else 6
            n = wc * DL
            blk = blocks[bb, cc, :m, :n].reshape(CO, 8, wbo, wc, DL)
            slab[:, :, bb * 4 : bb * 4 + wbo, cc * 8 : cc * 8 + wc, :] = blk
    return slab


# revision 22
# speedup vs baseline: 2.4091x; 1.9442x over previous
"""Trainium2 Bass kernel for 4D valid convolution.

x (2,2,32,32,64,64) f32, weight (4,2,3,3,3,3) f32, bias (4,) f32
-> out (2,4,30,30,62,62) f32  (valid cross-correlation + bias)

Strategy: 8 cores = batch(2) x a-quadrant(4). Each core computes
out[b, :, a_sel, :, :, :] from slab x[b, :, a0:a0+10, :, :, :].

TensorE mapping per core:
  K (contraction, partitions) = (ci=2, a-window=10, b-window=6) = 120
  M (psum partitions)         = (co=4, a_out=8, b_out=4) = 128
  N (streamed free dim)       = contiguous (c,d) output pixels, <=512
The weights for the (a,b) window band are prebuilt on host per (k,l)
tap; the 9 (k,l) taps accumulate in PSUM using (c,d)-shifted views of
the same SBUF x tile. float32r matmuls (1 row/cycle at N>=256).
"""

import sys

if "/opt/trn_rl_repo" not in sys.path:
    sys.path.insert(0, "/opt/trn_rl_repo")

import ml_dtypes
import numpy as np

BF16 = ml_dtypes.bfloat16

B, CI, CO = 2, 2, 4
A, B2, C, D = 32, 32, 64, 64
AO, BO, CL, DL = 30, 30, 62, 62
K = 3

# per-core a-slab starts; each core computes 8 output a-rows (q=3 overlaps q=2)
A0 = [0, 8, 16, 22]
SA = 10  # a-window (8 outputs + 2 halo)
SB = 6  # b-window per block (4 outputs + 2 halo)
NBB = 8  # b_out blocks: 7 full (4 wide) + 1 last (2 wide)
NCC = 8  # c chunks: 7 full (8 wide) + 1 last (6 wide)

_CACHE = {}


def _build_weights(weight: np.ndarray, bias: np.ndarray):
    """Banded lhsT matrices per (k,l) tap, plus per-partition bias vectors."""
    w = weight.astype(np.float32)

    def banded(sa, n_ao, sb, n_bo):
        # sel[d, o, t] = 1 if d == o + t
        sa_sel = np.zeros((sa, n_ao, K), np.float32)
        for t in range(K):
            for o in range(n_ao):
                sa_sel[o + t, o, t] = 1.0
        sb_sel = np.zeros((sb, n_bo, K), np.float32)
        for t in range(K):
            for o in range(n_bo):
                sb_sel[o + t, o, t] = 1.0
        # lhsT[t=(k,l)][(db,ci,da), (co,ao,bo)]  (partition order db-major)
        out = np.zeros((9, sb * CI * sa, CO * n_ao * n_bo), np.float32)
        for k in range(K):
            for l in range(K):
                wkl = w[:, :, :, :, k, l]  # (co, ci, i, j)
                m = np.einsum("dai,ebj,ocij->ecdoab", sa_sel, sb_sel, wkl)
                out[k * 3 + l] = m.reshape(sb * CI * sa, CO * n_ao * n_bo)
        return np.ascontiguousarray(out)

    w_main = banded(SA, 8, SB, 4)  # (9, 120, 128)
    w_last = banded(SA, 8, 4, 2)  # (9, 80, 64)
    bias_main = np.repeat(bias.astype(np.float32), 32).reshape(128, 1)
    bias_last = np.repeat(bias.astype(np.float32), 16).reshape(64, 1)
    return w_main, w_last, bias_main, bias_last


def _build_program():
    import concourse.bass as bass  # noqa: F401
    import concourse.mybir as mybir
    import concourse.tile as tile
    from concourse import bacc

    f32 = mybir.dt.float32
    bf16 = mybir.dt.bfloat16

    nc = bacc.Bacc("TRN2", target_bir_lowering=False, debug=False, num_devices=8)
    xs = nc.dram_tensor("x_slab", [CI, SA, B2, C, D], bf16, kind="ExternalInput")
    wm = nc.dram_tensor("w_main", [9, 120, 128], bf16, kind="ExternalInput")
    wl = nc.dram_tensor("w_last", [9, 80, 64], bf16, kind="ExternalInput")
    bm = nc.dram_tensor("bias_main", [128, 1], f32, kind="ExternalInput")
    bl = nc.dram_tensor("bias_last", [64, 1], f32, kind="ExternalInput")
    # partition-major blocks: [bb, cc, m, n]; host unscrambles (cheap numpy)
    out = nc.dram_tensor(
        "out_blocks", [NBB, NCC, 128, 8 * DL], f32, kind="ExternalOutput"
    )

    with tile.TileContext(nc) as tc:
        with (
            tc.tile_pool(name="w", bufs=1) as wpool,
            tc.tile_pool(name="rhs", bufs=8) as rpool,
            tc.tile_pool(name="psum", bufs=8, space="PSUM") as ppool,
            tc.tile_pool(name="ot", bufs=4) as opool,
        ):
            w_main_t = wpool.tile([120, 9 * 128], bf16)
            for t in range(9):
                nc.sync.dma_start(w_main_t[:, t * 128 : (t + 1) * 128], wm[t])
            w_last_t = wpool.tile([80, 9 * 64], bf16)
            for t in range(9):
                nc.sync.dma_start(w_last_t[:, t * 64 : (t + 1) * 64], wl[t])
            bias_main_t = wpool.tile([128, 1], f32)
            nc.sync.dma_start(bias_main_t[:], bm[:])
            bias_last_t = wpool.tile([64, 1], f32)
            nc.sync.dma_start(bias_last_t[:], bl[:])

            for bb in range(NBB):
                b0 = bb * 4
                wb = SB if bb < NBB - 1 else 4  # b-window width
                wbo = 4 if bb < NBB - 1 else 2  # b_out width
                P = CI * SA * wb  # 120 or 80
                M = CO * 8 * wbo  # 128 or 64

                rhs_t = rpool.tile([P, C * D], bf16, tag="rhs")
                for db in range(wb):
                    nc.sync.dma_start(
                        rhs_t[db * 20 : (db + 1) * 20, :],
                        xs[:, :, b0 + db].rearrange("ci a c d -> (ci a) (c d)"),
                    )
                rhs3 = rhs_t.rearrange("p (c d) -> p c d", c=C)
                w_t = w_main_t if bb < NBB - 1 else w_last_t
                bias_t = bias_main_t if bb < NBB - 1 else bias_last_t

                for cc in range(NCC):
                    c0 = cc * 8
                    wc = 8 if cc < NCC - 1 else 6
                    N = wc * DL
                    ps = ppool.tile([M, N], f32, tag="ps")
                    for t in range(9):
                        k, l = divmod(t, 3)
                        rv = rhs3[:, c0 + k : c0 + k + wc, l : l + DL]
                        nc.tensor.matmul(
                            ps.rearrange("m (c d) -> m c d", c=wc),
                            w_t[:, t * M : (t + 1) * M],
                            rv,
                            start=(t == 0),
                            stop=(t == 8),
                        )
                    ot = opool.tile([M, N], f32, tag="ot")
                    nc.vector.tensor_scalar_add(ot[:], ps[:], bias_t[:])
                    # store from the ACT queue: Sync stays free for loads
                    nc.scalar.dma_start(out[bb, cc, :M, :N], ot[:])
    nc.compile()
    return nc


def kernel(x: np.ndarray, weight: np.ndarray, bias: np.ndarray) -> np.ndarray:
    from concourse.bass_utils import run_bass_kernel_spmd

    if "nc" not in _CACHE:
        _CACHE["nc"] = _build_program()
    nc = _CACHE["nc"]

    w_main, w_last, bias_main, bias_last = _build_weights(weight, bias)
    x_bf = x.astype(BF16)
    w_main = w_main.astype(BF16)
    w_last = w_last.astype(BF16)

    in_maps = []
    for core in range(8):
        b, q = divmod(core, 4)
        a0 = A0[q]
        in_maps.append(
            {
                "x_slab": np.ascontiguousarray(x_bf[b, :, a0 : a0 + SA]),
                "w_main": w_main,
                "w_last": w_last,
                "bias_main": bias_main,
                "bias_last": bias_last,
            }
        )

    res = run_bass_kernel_spmd(nc, in_maps, core_ids=list(range(8)))
    _CACHE["last_result"] = res

    out = np.empty((B, CO, AO, BO, CL, DL), np.float32)
    for core in range(8):
        b, q = divmod(core, 4)
        slab = _unscramble(res.results[core]["out_blocks"])  # (4, 8, 30, 62, 62)
        if q < 3:
            out[b, :, 8 * q : 8 * q + 8] = slab
        else:
            out[b, :, 24:30] = slab[:, 2:8]
    return out


def _unscramble(blocks: np.ndarray) -> np.ndarray:
    """[NBB, NCC, 128, 8*62] partition-major blocks -> (4, 8, 30, 62, 62) slab."""
    slab = np.empty((CO, 8, BO, CL, DL), np.float32)
    for bb in range(NBB):
        wbo = 4 if bb < NBB - 1 else 2
        m = CO * 8 * wbo
        for cc in range(NCC):
            wc = 8 if cc < NCC - 1 else 6
            n = wc * DL
            blk = blocks[bb, cc, :m, :n].reshape(CO, 8, wbo, wc, DL)
            slab[:, :, bb * 4 : bb * 4 + wbo, cc * 8 : cc * 8 + wc, :] = blk
    return slab


# revision 25
# speedup vs baseline: 2.5920x; 1.0759x over previous
"""Trainium2 Bass kernel for 4D valid convolution.

x (2,2,32,32,64,64) f32, weight (4,2,3,3,3,3) f32, bias (4,) f32
-> out (2,4,30,30,62,62) f32  (valid cross-correlation + bias)

Strategy: 8 cores = batch(2) x a-quadrant(4). Each core computes
out[b, :, a_sel, :, :, :] from slab x[b, :, a0:a0+10, :, :, :].

TensorE mapping per core:
  K (contraction, partitions) = (ci=2, a-window=10, b-window=6) = 120
  M (psum partitions)         = (co=4, a_out=8, b_out=4) = 128
  N (streamed free dim)       = contiguous (c,d) output pixels, <=512
The weights for the (a,b) window band are prebuilt on host per (k,l)
tap; the 9 (k,l) taps accumulate in PSUM using (c,d)-shifted views of
the same SBUF x tile. float32r matmuls (1 row/cycle at N>=256).
"""

import sys

if "/opt/trn_rl_repo" not in sys.path:
    sys.path.insert(0, "/opt/trn_rl_repo")

import ml_dtypes
import numpy as np

BF16 = ml_dtypes.bfloat16

B, CI, CO = 2, 2, 4
A, B2, C, D = 32, 32, 64, 64
AO, BO, CL, DL = 30, 30, 62, 62
K = 3

# per-core a-slab starts; each core computes 8 output a-rows (q=3 overlaps q=2)
A0 = [0, 8, 16, 22]
SA = 10  # a-window (8 outputs + 2 halo)
SB = 6  # b-window per block (4 outputs + 2 halo)
NBB = 8  # b_out blocks: 7 full (4 wide) + 1 last (2 wide)
NCC = 8  # c chunks: 7 full (8 wide) + 1 last (6 wide)

_CACHE = {}


def _build_weights(weight: np.ndarray, bias: np.ndarray):
    """Banded lhsT matrices per (k,l) tap, plus per-partition bias vectors."""
    w = weight.astype(np.float32)

    def banded(sa, n_ao, sb, n_bo):
        # sel[d, o, t] = 1 if d == o + t
        sa_sel = np.zeros((sa, n_ao, K), np.float32)
        for t in range(K):
            for o in range(n_ao):
                sa_sel[o + t, o, t] = 1.0
        sb_sel = np.zeros((sb, n_bo, K), np.float32)
        for t in range(K):
            for o in range(n_bo):
                sb_sel[o + t, o, t] = 1.0
        # lhsT[(db,ci,da), t=(k,l), (co,ao,bo)] — taps side by side in columns
        # so the whole thing loads with a single 2D DMA into [P, 9*M]
        out = np.zeros((sb * CI * sa, 9, CO * n_ao * n_bo), np.float32)
        for k in range(K):
            for l in range(K):
                wkl = w[:, :, :, :, k, l]  # (co, ci, i, j)
                m = np.einsum("dai,ebj,ocij->ecdoab", sa_sel, sb_sel, wkl)
                out[:, k * 3 + l, :] = m.reshape(sb * CI * sa, CO * n_ao * n_bo)
        return np.ascontiguousarray(out.reshape(sb * CI * sa, 9 * CO * n_ao * n_bo))

    w_main = banded(SA, 8, SB, 4)  # (9, 120, 128)
    w_last = banded(SA, 8, 4, 2)  # (9, 80, 64)
    bias_main = np.repeat(bias.astype(np.float32), 32).reshape(128, 1)
    bias_last = np.repeat(bias.astype(np.float32), 16).reshape(64, 1)
    return w_main, w_last, bias_main, bias_last


def _build_program():
    import concourse.bass as bass  # noqa: F401
    import concourse.mybir as mybir
    import concourse.tile as tile
    from concourse import bacc

    f32 = mybir.dt.float32
    bf16 = mybir.dt.bfloat16

    nc = bacc.Bacc("TRN2", target_bir_lowering=False, debug=False, num_devices=8)
    xs = nc.dram_tensor("x_slab", [CI, SA, B2, C, D], bf16, kind="ExternalInput")
    wm = nc.dram_tensor("w_main", [120, 9 * 128], bf16, kind="ExternalInput")
    wl = nc.dram_tensor("w_last", [80, 9 * 64], bf16, kind="ExternalInput")
    bm = nc.dram_tensor("bias_main", [128, 1], f32, kind="ExternalInput")
    bl = nc.dram_tensor("bias_last", [64, 1], f32, kind="ExternalInput")
    # partition-major blocks: [bb, cc, m, n]; host unscrambles (cheap numpy)
    out = nc.dram_tensor(
        "out_blocks", [NBB, NCC, 128, 8 * DL], f32, kind="ExternalOutput"
    )

    with tile.TileContext(nc) as tc:
        with (
            tc.tile_pool(name="w", bufs=1) as wpool,
            tc.tile_pool(name="rhs", bufs=8) as rpool,
            tc.tile_pool(name="psum", bufs=8, space="PSUM") as ppool,
            tc.tile_pool(name="ot", bufs=4) as opool,
        ):
            w_main_t = wpool.tile([120, 9 * 128], bf16)
            nc.sync.dma_start(w_main_t[:], wm[:])
            w_last_t = wpool.tile([80, 9 * 64], bf16)
            nc.sync.dma_start(w_last_t[:], wl[:])
            bias_main_t = wpool.tile([128, 1], f32)
            nc.sync.dma_start(bias_main_t[:], bm[:])
            bias_last_t = wpool.tile([64, 1], f32)
            nc.sync.dma_start(bias_last_t[:], bl[:])

            for bb in range(NBB):
                b0 = bb * 4
                wb = SB if bb < NBB - 1 else 4  # b-window width
                wbo = 4 if bb < NBB - 1 else 2  # b_out width
                P = CI * SA * wb  # 120 or 80
                M = CO * 8 * wbo  # 128 or 64

                rhs_t = rpool.tile([P, C * D], bf16, tag="rhs")
                for db in range(wb):
                    nc.sync.dma_start(
                        rhs_t[db * 20 : (db + 1) * 20, :],
                        xs[:, :, b0 + db].rearrange("ci a c d -> (ci a) (c d)"),
                    )
                rhs3 = rhs_t.rearrange("p (c d) -> p c d", c=C)
                w_t = w_main_t if bb < NBB - 1 else w_last_t
                bias_t = bias_main_t if bb < NBB - 1 else bias_last_t

                for cc in range(NCC):
                    c0 = cc * 8
                    wc = 8 if cc < NCC - 1 else 6
                    N = wc * DL
                    ps = ppool.tile([M, N], f32, tag="ps")
                    for t in range(9):
                        k, l = divmod(t, 3)
                        rv = rhs3[:, c0 + k : c0 + k + wc, l : l + DL]
                        nc.tensor.matmul(
                            ps.rearrange("m (c d) -> m c d", c=wc),
                            w_t[:, t * M : (t + 1) * M],
                            rv,
                            start=(t == 0),
                            stop=(t == 8),
                        )
                    ot = opool.tile([M, N], f32, tag="ot")
                    nc.vector.tensor_scalar_add(ot[:], ps[:], bias_t[:])
                    # store from the ACT queue: Sync stays free for loads
                    nc.scalar.dma_start(out[bb, cc, :M, :N], ot[:])
    nc.compile()
    return nc


def kernel(x: np.ndarray, weight: np.ndarray, bias: np.ndarray) -> np.ndarray:
    from concourse.bass_utils import run_bass_kernel_spmd

    if "nc" not in _CACHE:
        _CACHE["nc"] = _build_program()
    nc = _CACHE["nc"]

    w_main, w_last, bias_main, bias_last = _build_weights(weight, bias)
    x_bf = x.astype(BF16)
    w_main = w_main.astype(BF16)
    w_last = w_last.astype(BF16)

    in_maps = []
    for core in range(8):
        b, q = divmod(core, 4)
        a0 = A0[q]
        in_maps.append(
            {
                "x_slab": np.ascontiguousarray(x_bf[b, :, a0 : a0 + SA]),
                "w_main": w_main,
                "w_last": w_last,
                "bias_main": bias_main,
                "bias_last": bias_last,
            }
        )

    res = run_bass_kernel_spmd(nc, in_maps, core_ids=list(range(8)))
    _CACHE["last_result"] = res

    out = np.empty((B, CO, AO, BO, CL, DL), np.float32)
    for core in range(8):
        b, q = divmod(core, 4)
        slab = _unscramble(res.results[core]["out_blocks"])  # (4, 8, 30, 62, 62)
        if q < 3:
            out[b, :, 8 * q : 8 * q + 8] = slab
        else:
            out[b, :, 24:30] = slab[:, 2:8]
    return out


def _unscramble(blocks: np.ndarray) -> np.ndarray:
    """[NBB, NCC, 128, 8*62] partition-major blocks -> (4, 8, 30, 62, 62) slab."""
    slab = np.empty((CO, 8, BO, CL, DL), np.float32)
    for bb in range(NBB):
        wbo = 4 if bb < NBB - 1 else 2
        m = CO * 8 * wbo
        for cc in range(NCC):
            wc = 8 if cc < NCC - 1 else 6
            n = wc * DL
            blk = blocks[bb, cc, :m, :n].reshape(CO, 8, wbo, wc, DL)
            slab[:, :, bb * 4 : bb * 4 + wbo, cc * 8 : cc * 8 + wc, :] = blk
    return slab


# revision 26
# speedup vs baseline: 2.6090x; 1.0066x over previous
"""Trainium2 Bass kernel for 4D valid convolution.

x (2,2,32,32,64,64) f32, weight (4,2,3,3,3,3) f32, bias (4,) f32
-> out (2,4,30,30,62,62) f32  (valid cross-correlation + bias)

Strategy: 8 cores = batch(2) x a-quadrant(4). Each core computes
out[b, :, a_sel, :, :, :] from slab x[b, :, a0:a0+10, :, :, :].

TensorE mapping per core:
  K (contraction, partitions) = (ci=2, a-window=10, b-window=6) = 120
  M (psum partitions)         = (co=4, a_out=8, b_out=4) = 128
  N (streamed free dim)       = contiguous (c,d) output pixels, <=512
The weights for the (a,b) window band are prebuilt on host per (k,l)
tap; the 9 (k,l) taps accumulate in PSUM using (c,d)-shifted views of
the same SBUF x tile. float32r matmuls (1 row/cycle at N>=256).
"""

import sys

if "/opt/trn_rl_repo" not in sys.path:
    sys.path.insert(0, "/opt/trn_rl_repo")

import ml_dtypes
import numpy as np

BF16 = ml_dtypes.bfloat16

B, CI, CO = 2, 2, 4
A, B2, C, D = 32, 32, 64, 64
AO, BO, CL, DL = 30, 30, 62, 62
K = 3

# per-core a-slab starts; each core computes 8 output a-rows (q=3 overlaps q=2)
A0 = [0, 8, 16, 22]
SA = 10  # a-window (8 outputs + 2 halo)
SB = 6  # b-window per block (4 outputs + 2 halo)
NBB = 8  # b_out blocks: 7 full (4 wide) + 1 last (2 wide)
NCC = 8  # c chunks: 7 full (8 wide) + 1 last (6 wide)

_CACHE = {}


def _build_weights(weight: np.ndarray, bias: np.ndarray):
    """Banded lhsT matrices per (k,l) tap, plus per-partition bias vectors."""
    w = weight.astype(np.float32)

    def banded(sa, n_ao, sb, n_bo):
        # sel[d, o, t] = 1 if d == o + t
        sa_sel = np.zeros((sa, n_ao, K), np.float32)
        for t in range(K):
            for o in range(n_ao):
                sa_sel[o + t, o, t] = 1.0
        sb_sel = np.zeros((sb, n_bo, K), np.float32)
        for t in range(K):
            for o in range(n_bo):
                sb_sel[o + t, o, t] = 1.0
        # lhsT[(db,ci,da), t=(k,l), (co,ao,bo)] — taps side by side in columns
        # so the whole thing loads with a single 2D DMA into [P, 9*M]
        out = np.zeros((sb * CI * sa, 9, CO * n_ao * n_bo), np.float32)
        for k in range(K):
            for l in range(K):
                wkl = w[:, :, :, :, k, l]  # (co, ci, i, j)
                m = np.einsum("dai,ebj,ocij->ecdoab", sa_sel, sb_sel, wkl)
                out[:, k * 3 + l, :] = m.reshape(sb * CI * sa, CO * n_ao * n_bo)
        return np.ascontiguousarray(out.reshape(sb * CI * sa, 9 * CO * n_ao * n_bo))

    w_main = banded(SA, 8, SB, 4)  # (9, 120, 128)
    w_last = banded(SA, 8, 4, 2)  # (9, 80, 64)
    bias_main = np.repeat(bias.astype(np.float32), 32).reshape(128, 1)
    bias_last = np.repeat(bias.astype(np.float32), 16).reshape(64, 1)
    return w_main, w_last, bias_main, bias_last


def _build_program():
    import concourse.bass as bass  # noqa: F401
    import concourse.mybir as mybir
    import concourse.tile as tile
    from concourse import bacc

    f32 = mybir.dt.float32
    bf16 = mybir.dt.bfloat16

    nc = bacc.Bacc("TRN2", target_bir_lowering=False, debug=False, num_devices=8)
    xs = nc.dram_tensor("x_slab", [CI, SA, B2, C, D], bf16, kind="ExternalInput")
    wm = nc.dram_tensor("w_main", [120, 9 * 128], bf16, kind="ExternalInput")
    wl = nc.dram_tensor("w_last", [80, 9 * 64], bf16, kind="ExternalInput")
    bm = nc.dram_tensor("bias_main", [128, 1], f32, kind="ExternalInput")
    bl = nc.dram_tensor("bias_last", [64, 1], f32, kind="ExternalInput")
    # partition-major blocks: [bb, cc, m, n]; host unscrambles (cheap numpy)
    out = nc.dram_tensor(
        "out_blocks", [NBB, NCC, 128, 8 * DL], f32, kind="ExternalOutput"
    )

    with tile.TileContext(nc) as tc:
        with (
            tc.tile_pool(name="w", bufs=1) as wpool,
            tc.tile_pool(name="rhs", bufs=8) as rpool,
            tc.tile_pool(name="psum", bufs=8, space="PSUM") as ppool,
            tc.tile_pool(name="ot", bufs=4) as opool,
        ):
            w_main_t = wpool.tile([120, 9 * 128], bf16)
            nc.sync.dma_start(w_main_t[:], wm[:])
            # first block's rhs next in the Sync queue so MMs start ASAP;
            # w_last/bias are only needed later, issue via ACT queue
            w_last_t = wpool.tile([80, 9 * 64], bf16)
            bias_main_t = wpool.tile([128, 1], f32)
            bias_last_t = wpool.tile([64, 1], f32)
            nc.scalar.dma_start(bias_main_t[:], bm[:])
            nc.scalar.dma_start(w_last_t[:], wl[:])
            nc.scalar.dma_start(bias_last_t[:], bl[:])

            for bb in range(NBB):
                b0 = bb * 4
                wb = SB if bb < NBB - 1 else 4  # b-window width
                wbo = 4 if bb < NBB - 1 else 2  # b_out width
                P = CI * SA * wb  # 120 or 80
                M = CO * 8 * wbo  # 128 or 64

                rhs_t = rpool.tile([P, C * D], bf16, tag="rhs")
                for db in range(wb):
                    nc.sync.dma_start(
                        rhs_t[db * 20 : (db + 1) * 20, :],
                        xs[:, :, b0 + db].rearrange("ci a c d -> (ci a) (c d)"),
                    )
                rhs3 = rhs_t.rearrange("p (c d) -> p c d", c=C)
                w_t = w_main_t if bb < NBB - 1 else w_last_t
                bias_t = bias_main_t if bb < NBB - 1 else bias_last_t

                for cc in range(NCC):
                    c0 = cc * 8
                    wc = 8 if cc < NCC - 1 else 6
                    N = wc * DL
                    ps = ppool.tile([M, N], f32, tag="ps")
                    for t in range(9):
                        k, l = divmod(t, 3)
                        rv = rhs3[:, c0 + k : c0 + k + wc, l : l + DL]
                        nc.tensor.matmul(
                            ps.rearrange("m (c d) -> m c d", c=wc),
                            w_t[:, t * M : (t + 1) * M],
                            rv,
                            start=(t == 0),
                            stop=(t == 8),
                        )
                    ot = opool.tile([M, N], f32, tag="ot")
                    nc.vector.tensor_scalar_add(ot[:], ps[:], bias_t[:])
                    # store from the ACT queue: Sync stays free for loads
                    nc.scalar.dma_start(out[bb, cc, :M, :N], ot[:])
    nc.compile()
    return nc


def kernel(x: np.ndarray, weight: np.ndarray, bias: np.ndarray) -> np.ndarray:
    from concourse.bass_utils import run_bass_kernel_spmd

    if "nc" not in _CACHE:
        _CACHE["nc"] = _build_program()
    nc = _CACHE["nc"]

    w_main, w_last, bias_main, bias_last = _build_weights(weight, bias)
    x_bf = x.astype(BF16)
    w_main = w_main.astype(BF16)
    w_last = w_last.astype(BF16)

    in_maps = []
    for core in range(8):
        b, q = divmod(core, 4)
        a0 = A0[q]
        in_maps.append(
            {
                "x_slab": np.ascontiguousarray(x_bf[b, :, a0 : a0 + SA]),
                "w_main": w_main,
                "w_last": w_last,
                "bias_main": bias_main,
                "bias_last": bias_last,
            }
        )

    res = run_bass_kernel_spmd(nc, in_maps, core_ids=list(range(8)))
    _CACHE["last_result"] = res

    out = np.empty((B, CO, AO, BO, CL, DL), np.float32)
    for core in range(8):
        b, q = divmod(core, 4)
        slab = _unscramble(res.results[core]["out_blocks"])  # (4, 8, 30, 62, 62)
        if q < 3:
            out[b, :, 8 * q : 8 * q + 8] = slab
        else:
            out[b, :, 24:30] = slab[:, 2:8]
    return out


def _unscramble(blocks: np.ndarray) -> np.ndarray:
    """[NBB, NCC, 128, 8*62] partition-major blocks -> (4, 8, 30, 62, 62) slab."""
    slab = np.empty((CO, 8, BO, CL, DL), np.float32)
    for bb in range(NBB):
        wbo = 4 if bb < NBB - 1 else 2
        m = CO * 8 * wbo
        for cc in range(NCC):
            wc = 8 if cc < NCC - 1 else 6
            n = wc * DL
            blk = blocks[bb, cc, :m, :n].reshape(CO, 8, wbo, wc, DL)
            slab[:, :, bb * 4 : bb * 4 + wbo, cc * 8 : cc * 8 + wc, :] = blk
    return slab


# revision 27
# speedup vs baseline: 2.6158x; 1.0026x over previous
"""Trainium2 Bass kernel for 4D valid convolution.

x (2,2,32,32,64,64) f32, weight (4,2,3,3,3,3) f32, bias (4,) f32
-> out (2,4,30,30,62,62) f32  (valid cross-correlation + bias)

Strategy: 8 cores = batch(2) x a-quadrant(4). Each core computes
out[b, :, a_sel, :, :, :] from slab x[b, :, a0:a0+10, :, :, :].

TensorE mapping per core (bf16 inputs, f32 PSUM accumulate):
  K (contraction, partitions) = (b-window=6, ci=2, a-window=10) = 120
  M (psum partitions)         = (co=4, a_out=8, b_out=4) = 128
  N (streamed free dim)       = contiguous (c,d) output pixels, <=496
Host prebuilds banded lhsT matrices (one per (k,l) tap, side by side in
one [120, 9*128] array -> a single DMA); the 9 (k,l) taps accumulate in
PSUM using (c,d)-shifted views of the same SBUF x tile, so each weight
load serves a full 496-column stream and the PE runs back-to-back at
~N cycles/matmul. Loads issue from the Sync DGE queue, stores from the
ACT queue (keeps Sync free to prefetch), evictions (bias add) on DVE.
Output goes to DRAM partition-major per (b-block, c-chunk); the host
unscrambles (SBUF-side multi-dim partition DMAs mislower, so the device
only ever does flat [P, N] stores).

Measured: ~145 us HW exec (8 cores), max rel err ~2.2e-3 vs f32
reference (bf16 input rounding; PE pitch ~210 ns/matmul = bf16
streaming roofline for this shape).
"""

import sys

if "/opt/trn_rl_repo" not in sys.path:
    sys.path.insert(0, "/opt/trn_rl_repo")

import ml_dtypes
import numpy as np

BF16 = ml_dtypes.bfloat16

B, CI, CO = 2, 2, 4
A, B2, C, D = 32, 32, 64, 64
AO, BO, CL, DL = 30, 30, 62, 62
K = 3

# per-core a-slab starts; each core computes 8 output a-rows (q=3 overlaps q=2)
A0 = [0, 8, 16, 22]
SA = 10  # a-window (8 outputs + 2 halo)
SB = 6  # b-window per block (4 outputs + 2 halo)
NBB = 8  # b_out blocks: 7 full (4 wide) + 1 last (2 wide)
NCC = 8  # c chunks: 7 full (8 wide) + 1 last (6 wide)

_CACHE = {}


def _build_weights(weight: np.ndarray, bias: np.ndarray):
    """Banded lhsT matrices per (k,l) tap, plus per-partition bias vectors."""
    w = weight.astype(np.float32)

    def banded(sa, n_ao, sb, n_bo):
        # sel[d, o, t] = 1 if d == o + t
        sa_sel = np.zeros((sa, n_ao, K), np.float32)
        for t in range(K):
            for o in range(n_ao):
                sa_sel[o + t, o, t] = 1.0
        sb_sel = np.zeros((sb, n_bo, K), np.float32)
        for t in range(K):
            for o in range(n_bo):
                sb_sel[o + t, o, t] = 1.0
        # lhsT[(db,ci,da), t=(k,l), (co,ao,bo)] — taps side by side in columns
        # so the whole thing loads with a single 2D DMA into [P, 9*M]
        out = np.zeros((sb * CI * sa, 9, CO * n_ao * n_bo), np.float32)
        for k in range(K):
            for l in range(K):
                wkl = w[:, :, :, :, k, l]  # (co, ci, i, j)
                m = np.einsum("dai,ebj,ocij->ecdoab", sa_sel, sb_sel, wkl)
                out[:, k * 3 + l, :] = m.reshape(sb * CI * sa, CO * n_ao * n_bo)
        return np.ascontiguousarray(out.reshape(sb * CI * sa, 9 * CO * n_ao * n_bo))

    w_main = banded(SA, 8, SB, 4)  # (9, 120, 128)
    w_last = banded(SA, 8, 4, 2)  # (9, 80, 64)
    bias_main = np.repeat(bias.astype(np.float32), 32).reshape(128, 1)
    bias_last = np.repeat(bias.astype(np.float32), 16).reshape(64, 1)
    return w_main, w_last, bias_main, bias_last


def _build_program():
    import concourse.bass as bass  # noqa: F401
    import concourse.mybir as mybir
    import concourse.tile as tile
    from concourse import bacc

    f32 = mybir.dt.float32
    bf16 = mybir.dt.bfloat16

    nc = bacc.Bacc("TRN2", target_bir_lowering=False, debug=False, num_devices=8)
    xs = nc.dram_tensor("x_slab", [CI, SA, B2, C, D], bf16, kind="ExternalInput")
    wm = nc.dram_tensor("w_main", [120, 9 * 128], bf16, kind="ExternalInput")
    wl = nc.dram_tensor("w_last", [80, 9 * 64], bf16, kind="ExternalInput")
    bm = nc.dram_tensor("bias_main", [128, 1], f32, kind="ExternalInput")
    bl = nc.dram_tensor("bias_last", [64, 1], f32, kind="ExternalInput")
    # partition-major blocks: [bb, cc, m, n]; host unscrambles (cheap numpy)
    out = nc.dram_tensor(
        "out_blocks", [NBB, NCC, 128, 8 * DL], f32, kind="ExternalOutput"
    )

    with tile.TileContext(nc) as tc:
        with (
            tc.tile_pool(name="w", bufs=1) as wpool,
            tc.tile_pool(name="rhs", bufs=8) as rpool,
            tc.tile_pool(name="psum", bufs=8, space="PSUM") as ppool,
            tc.tile_pool(name="ot", bufs=4) as opool,
        ):
            w_main_t = wpool.tile([120, 9 * 128], bf16)
            nc.sync.dma_start(w_main_t[:], wm[:])
            # first block's rhs next in the Sync queue so MMs start ASAP;
            # w_last/bias are only needed later, issue via ACT queue
            w_last_t = wpool.tile([80, 9 * 64], bf16)
            bias_main_t = wpool.tile([128, 1], f32)
            bias_last_t = wpool.tile([64, 1], f32)
            nc.scalar.dma_start(bias_main_t[:], bm[:])
            nc.scalar.dma_start(w_last_t[:], wl[:])
            nc.scalar.dma_start(bias_last_t[:], bl[:])

            for bb in range(NBB):
                b0 = bb * 4
                wb = SB if bb < NBB - 1 else 4  # b-window width
                wbo = 4 if bb < NBB - 1 else 2  # b_out width
                P = CI * SA * wb  # 120 or 80
                M = CO * 8 * wbo  # 128 or 64

                rhs_t = rpool.tile([P, C * D], bf16, tag="rhs")
                for db in range(wb):
                    nc.sync.dma_start(
                        rhs_t[db * 20 : (db + 1) * 20, :],
                        xs[:, :, b0 + db].rearrange("ci a c d -> (ci a) (c d)"),
                    )
                rhs3 = rhs_t.rearrange("p (c d) -> p c d", c=C)
                w_t = w_main_t if bb < NBB - 1 else w_last_t
                bias_t = bias_main_t if bb < NBB - 1 else bias_last_t

                for cc in range(NCC):
                    c0 = cc * 8
                    wc = 8 if cc < NCC - 1 else 6
                    N = wc * DL
                    ps = ppool.tile([M, N], f32, tag="ps")
                    for t in range(9):
                        k, l = divmod(t, 3)
                        rv = rhs3[:, c0 + k : c0 + k + wc, l : l + DL]
                        nc.tensor.matmul(
                            ps.rearrange("m (c d) -> m c d", c=wc),
                            w_t[:, t * M : (t + 1) * M],
                            rv,
                            start=(t == 0),
                            stop=(t == 8),
                        )
                    ot = opool.tile([M, N], f32, tag="ot")
                    nc.vector.tensor_scalar_add(ot[:], ps[:], bias_t[:])
                    # store from the ACT queue: Sync stays free for loads
                    nc.scalar.dma_start(out[bb, cc, :M, :N], ot[:])
    nc.compile()
    return nc


def kernel(x: np.ndarray, weight: np.ndarray, bias: np.ndarray) -> np.ndarray:
    from concourse.bass_utils import run_bass_kernel_spmd

    if "nc" not in _CACHE:
        _CACHE["nc"] = _build_program()
    nc = _CACHE["nc"]

    w_main, w_last, bias_main, bias_last = _build_weights(weight, bias)
    x_bf = x.astype(BF16)
    w_main = w_main.astype(BF16)
    w_last = w_last.astype(BF16)

    in_maps = []
    for core in range(8):
        b, q = divmod(core, 4)
        a0 = A0[q]
        in_maps.append(
            {
                "x_slab": np.ascontiguousarray(x_bf[b, :, a0 : a0 + SA]),
                "w_main": w_main,
                "w_last": w_last,
                "bias_main": bias_main,
                "bias_last": bias_last,
            }
        )

    res = run_bass_kernel_spmd(nc, in_maps, core_ids=list(range(8)))
    _CACHE["last_result"] = res

    out = np.empty((B, CO, AO, BO, CL, DL), np.float32)
    for core in range(8):
        b, q = divmod(core, 4)
        slab = _unscramble(res.results[core]["out_blocks"])  # (4, 8, 30, 62, 62)
        if q < 3:
            out[b, :, 8 * q : 8 * q + 8] = slab
        else:
            out[b, :, 24:30] = slab[:, 2:8]
    return out


def _unscramble(blocks: np.ndarray) -> np.ndarray:
    """[NBB, NCC, 128, 8*62] partition-major blocks -> (4, 8, 30, 62, 62) slab."""
    slab = np.empty((CO, 8, BO, CL, DL), np.float32)
    for bb in range(NBB):
        wbo = 4 if bb < NBB - 1 else 2
        m = CO * 8 * wbo
        for cc in range(NCC):
            wc = 8 if cc < NCC - 1 else 6
            n = wc * DL
            blk = blocks[bb, cc, :m, :n].reshape(CO, 8, wbo, wc, DL)
            slab[:, :, bb * 4 : bb * 4 + wbo, cc * 8 : cc * 8 + wc, :] = blk
    return slab
